# revision 30
# baseline (speedup 1.0000x reference)
# Masked multi-head attention for Trainium2, SPMD over 8 NeuronCores.
#
# Problem: q,k,v [2,16,2048,64] f32, mask [1,1,2048,2048] int32 (0/1),
#   out[b,h] = softmax(q@k^T/8 masked) @ v.
#
# Sharding: B*H = 32 heads, 4 per core (embarrassingly parallel).
#
# Per-head on-chip algorithm (no max-subtraction needed: scores ~ N(0,1),
# exp never overflows fp32; masked softmax == exp(S)*mask / sum(exp(S)*mask)):
#   Work in the transposed orientation S^T[k,q] so the softmax reduction
#   (over k) lands on the PE contraction dim instead of needing a
#   partition-axis reduction:
#     S^T[kc] (psum)  = kT[:,kc].T @ qT            (kc = 16 chunks of 128 k)
#     P^T[kc] (sbuf)  = exp(S^T[kc])          [ScalarE, psum->sbuf, bf16]
#     P^T[kc]        *= maskT[kc]             [VectorE, bf16 2x mode]
#     acc[65,2048]   += vp[kc].T @ P^T[kc]    [vp = [V | ones], fp32 psum]
#   acc rows 0..63 = (P@V)^T, row 64 = l = sum_k P.  Host divides and
#   transposes back.  The 1/sqrt(64) scale is folded into qT on the host.
import os
from contextlib import ExitStack

import numpy as np
import ml_dtypes

B, H, S, D = 2, 16, 2048, 64
N_CORES = 8
HPC = (B * H) // N_CORES  # heads per core = 4
P = 128
NCHUNK = S // P  # 16

# Precision / tiling knobs.  NOTE: TRN2 matmul output must be fp32, so score
# psum is always f32: span=1024 keeps psum at 2(st)x2bufs + 4(acc) = 8 banks.
# fp16 measured ~6x more accurate than bf16 at identical HW speed
# (rel absmax 8.2e-4 vs 4.7e-3 across all heads).
PREC = "fp16"  # 16-bit dtype for qT/kT/vp/maskT/P ("fp16" | "bf16")
QK_DT = "16"  # dtype of qT/kT fed to the PE ("16" | "f32r" | "f32")
QK_PACK = False  # row-tile the QK matmuls: 2 chunks concurrently (d=64 pairs)
MASK_SEP = False  # mask-multiply into a separate tile instead of in-place
PV_DELAY = 3  # software-pipeline depth: emit chunk c's PV after QK of c+PV_DELAY
PT_BUFS = 8  # pt pool slots (ACT->DVE->PE pipeline depth)
IO_BUFS = 4  # per-head qT/kT/vp prefetch depth (all heads resident)
MASK_GPS = False  # route every second mask-multiply to GpSimd (DVE relief)
PV_ILV = False  # interleave delayed PV per-span with QK instead of per-chunk
MASK_WIDE = False  # one FD-2048 mask TT per chunk (pt tile spans both spans)
LW_FILL = 0  # dummy ldweights per fill point (hold PE p-state ramp); 0 = off
LW_W = 128  # free-dim columns per dummy ldweights
COPY_GPS = True  # o DMA descriptor-gen on the idle GpSimd queue (frees SP)
IO_FIRST = True  # emit head-0/1 IO DMAs before mask DMAs (startup overlap)
# QK_DUP: feed the PE K=128 by duplicating the 64 d-rows of qT/kT host-side
# (x0.5 folded into the q scale) and split each score tile into two
# 64-partition column-tile matmuls.  Empirically (K=128, P_out=64, N=512)
# streams 1 col/cycle vs 2.75 for the (K=64, P_out=128) form - the PE is
# measured ~2.7x slower than the cost model there, and is the kernel's
# real bottleneck on HW.
QK_DUP = True
QK_KH_MAJOR = False  # pair same-stationary QK matmuls adjacently
# QK_QUAD: 2x2 PE quadrant tiling.  The PE executes matmuls on disjoint
# 64-row x 64-col array tiles CONCURRENTLY (measured 78.7 ns per
# [K=64 -> 64, N=512] matmul in a 4-quadrant rotation vs ~250 serial).
# QK: rows 64-127 hold host-duplicated q/k data (no scale change - each
# matmul still contracts K=64), and the two k-halves of each chunk rotate
# over (row-tile, col-tile) quadrants.  PV: the chunk contraction splits
# into k 0-63 (partitions 0-63, col-tile 0) and k 64-127 (row-tile 1)
# accumulating into two parity psum accs, merged by one DVE add at the end.
# Layout is per q-half (1024 cols) so st x2 + accA + accB fit in 8 banks.
QK_QUAD = False
PV_SPLIT = False  # split PV contraction over row-tile parity accs (quad PV)
# IVL2: interleave two heads' chunk pipelines.  In situ every matmul carries
# ~100-150ns of cross-engine sync latency; alternating two independent heads
# gives each engine work while the other head's dependencies settle.
# Per-(head, q-half) accs [65, span] keep PSUM at st(2x2) + acc(2x2) = 8 banks.
IVL2 = False
# ablation knobs (bench-only attribution experiments; break correctness)
MASK_OFF = False
PV_OFF = False
QK_OFF = False
MASK_DMA_ONCE = False  # bench-only: hoist mask DMA out of the For_i loop

_CACHE = {}
LAST_RESULT = None  # BassKernelResults of the most recent run (for test.py)


def _build_nc(loop_reps=None):
    """Build the Bass program.  loop_reps=None -> the real kernel;
    loop_reps=K wraps the whole body in a hardware For_i loop (bench-only:
    lets wall-clock diffs between two K values measure per-iteration HW
    time through the slow axon tunnel)."""
    import concourse.bass as bass
    import concourse.tile as tile
    from concourse import bacc, mybir

    DT16 = mybir.dt.float16 if PREC == "fp16" else mybir.dt.bfloat16
    F32 = mybir.dt.float32
    qk_mm_dt = {"16": DT16, "f32r": mybir.dt.float32r, "f32": F32}[QK_DT]
    score_dt = F32
    # one matmul output must fit in one psum bank (512 fp32/partition)
    qk_n = 512
    # one score tile: free-dim span of a single exp instruction
    span = 1024
    spans = S // span

    nc = bacc.Bacc("TRN2", target_bir_lowering=False, debug=False)

    qk_rows = 128 if (QK_PACK or QK_DUP or QK_QUAD or IVL2) else 64
    qT = nc.dram_tensor("qT", [HPC, qk_rows, S], qk_mm_dt, kind="ExternalInput").ap()
    kT = nc.dram_tensor("kT", [HPC, qk_rows, S], qk_mm_dt, kind="ExternalInput").ap()
    vp = nc.dram_tensor("vp", [HPC, S, D + 1], DT16, kind="ExternalInput").ap()
    maskT = nc.dram_tensor("maskT", [S, S], DT16, kind="ExternalInput").ap()
    o = nc.dram_tensor("o", [HPC, D + 1, S], F32, kind="ExternalOutput").ap()

    with tile.TileContext(nc) as tc, ExitStack() as ctx:
        mask_pool = ctx.enter_context(tc.tile_pool(name="mask", bufs=NCHUNK + 2))
        io_pool = ctx.enter_context(tc.tile_pool(name="io", bufs=IO_BUFS))
        pt_pool = ctx.enter_context(tc.tile_pool(name="pt", bufs=PT_BUFS))
        out_pool = ctx.enter_context(tc.tile_pool(name="outsb", bufs=2))
        qk_psum = ctx.enter_context(tc.tile_pool(name="qk_psum", bufs=2, space="PSUM"))
        acc_psum = ctx.enter_context(tc.tile_pool(name="acc_psum", bufs=1, space="PSUM"))

        def load_mask(cs=range(NCHUNK), tiles=None):
            # mask^T resident in SBUF for all heads, one tile per k-chunk
            # (bufs = NCHUNK+2 so a following iteration's reload can start
            # while late chunks of the previous one are still being read).
            mt = maskT.rearrange("(c p) q -> p c q", p=P)
            if tiles is None:
                tiles = [None] * NCHUNK
            for c in cs:
                mtile = mask_pool.tile([P, S], DT16, tag="mchunk", name=f"mask_c{c}")
                nc.sync.dma_start(mtile[:], mt[:, c, :])
                tiles[c] = mtile
            return tiles

        def load_io(h):
            qT_sb = io_pool.tile([qk_rows, S], qk_mm_dt, tag="qT", name=f"qT_sb{h}")
            nc.sync.dma_start(qT_sb[:], qT[h])
            kT_sb = io_pool.tile([qk_rows, S], qk_mm_dt, tag="kT", name=f"kT_sb{h}")
            nc.sync.dma_start(kT_sb[:], kT[h])
            vp_sb = io_pool.tile([P, NCHUNK, D + 1], DT16, tag="vp", name=f"vp_sb{h}")
            nc.sync.dma_start(vp_sb[:], vp[h].rearrange("(c p) d -> p c d", p=P))
            return qT_sb, kT_sb, vp_sb

        hoisted = [None]

        def body(_iv=None):
            if hoisted[0] is not None:
                _heads(hoisted[0], None)
            elif IO_FIRST:
                ios = [load_io(0)]
                maskT_sb = load_mask(range(0, 4))
                ios.append(load_io(1))
                load_mask(range(4, NCHUNK), maskT_sb)
                _heads(maskT_sb, ios)
            else:
                _heads(load_mask(), None)

        def _head_quad(h, qT_sb, kT_sb, vp_sb, maskT_sb):
            out_sb = out_pool.tile([D + 1, S], F32, tag="out", name=f"out_sb{h}")
            for qh in range(spans):
                q0 = qh * span
                accA = acc_psum.tile(
                    [D + 1, span], F32, tag="accA", name=f"accA{h}_{qh}", bufs=1
                )
                accB = None
                if PV_SPLIT:
                    accB = acc_psum.tile(
                        [D + 1, span], F32, tag="accB", name=f"accB{h}_{qh}", bufs=1
                    )

                def emit_pv_q(c, pt):
                    if PV_OFF:
                        return
                    for j in range(span // 512):
                        sl = slice(j * 512, (j + 1) * 512)
                        if not PV_SPLIT:
                            nc.tensor.matmul(
                                accA[:, sl],
                                lhsT=vp_sb[:, c, :],
                                rhs=pt[:, sl],
                                start=(c == 0),
                                stop=(c == NCHUNK - 1),
                            )
                            continue
                        nc.tensor.matmul(
                            accA[:, sl],
                            lhsT=vp_sb[0:64, c, :],
                            rhs=pt[0:64, sl],
                            start=(c == 0),
                            stop=(c == NCHUNK - 1),
                        )
                        nc.tensor.matmul(
                            accB[:, sl],
                            lhsT=vp_sb[64:128, c, :],
                            rhs=pt[64:128, sl],
                            start=(c == 0),
                            stop=(c == NCHUNK - 1),
                        )

                pending = []
                for c in range(NCHUNK):
                    st = qk_psum.tile(
                        [P, span], score_dt, tag="st", name=f"stq{h}_{qh}_{c}"
                    )
                    for j in range(span // 512):
                        jsl = slice(j * 512, (j + 1) * 512)
                        if QK_OFF:
                            continue
                        for r in range(2):
                            kh = r ^ (j % 2)  # rotate quadrants
                            nc.tensor.matmul(
                                st[64 * kh : 64 * kh + 64, jsl],
                                lhsT=kT_sb[
                                    64 * r : 64 * r + 64,
                                    c * P + 64 * kh : c * P + 64 * kh + 64,
                                ],
                                rhs=qT_sb[
                                    64 * r : 64 * r + 64, q0 + j * 512 : q0 + j * 512 + 512
                                ],
                                start=True,
                                stop=True,
                            )
                    pt = pt_pool.tile([P, span], DT16, tag="pt", name=f"ptq{h}_{qh}_{c}")
                    nc.scalar.activation(pt[:], st[:], mybir.ActivationFunctionType.Exp)
                    if not MASK_OFF:
                        nc.vector.tensor_mul(
                            pt[:], pt[:], maskT_sb[c][:, q0 : q0 + span]
                        )
                    pending.append((c, pt))
                    if len(pending) > PV_DELAY:
                        emit_pv_q(*pending.pop(0))
                for item in pending:
                    emit_pv_q(*item)
                if PV_OFF:
                    nc.gpsimd.memset(out_sb[:, q0 : q0 + span], 0.0)
                else:
                    # DVE may read only one PSUM operand per instruction:
                    # drain accA to SBUF, then add accB on top.
                    nc.vector.tensor_copy(out_sb[:, q0 : q0 + span], accA[:])
                    if PV_SPLIT:
                        nc.vector.tensor_add(
                            out_sb[:, q0 : q0 + span],
                            out_sb[:, q0 : q0 + span],
                            accB[:],
                        )
            (nc.gpsimd if COPY_GPS else nc.sync).dma_start(o[h], out_sb[:])

        def _heads_ivl2(maskT_sb, ios):
          for hp in range(0, HPC, 2):
            pair = []
            for h in (hp, hp + 1):
                if ios is not None and h < len(ios):
                    pair.append(ios[h])
                else:
                    pair.append(load_io(h))
            outs = [
                out_pool.tile([D + 1, S], F32, tag="out", name=f"out_sb{hp + i}")
                for i in range(2)
            ]
            for qh in range(spans):
                q0 = qh * span
                accs = [
                    acc_psum.tile(
                        [D + 1, span], F32, tag=f"acc{i}",
                        name=f"acc{hp + i}_{qh}", bufs=1,
                    )
                    for i in range(2)
                ]

                def emit_pv_i(i, c, pt):
                    if PV_OFF:
                        return
                    vp_sb = pair[i][2]
                    for j in range(span // 512):
                        sl = slice(j * 512, (j + 1) * 512)
                        nc.tensor.matmul(
                            accs[i][:, sl],
                            lhsT=vp_sb[:, c, :],
                            rhs=pt[:, sl],
                            start=(c == 0),
                            stop=(c == NCHUNK - 1),
                        )

                pend = []
                for c in range(NCHUNK):
                    pts = []
                    for i, (qT_sb, kT_sb, vp_sb) in enumerate(pair):
                        st = qk_psum.tile(
                            [P, span], score_dt, tag="st",
                            name=f"sti{hp + i}_{qh}_{c}",
                        )
                        if not QK_OFF:
                            for j in range(span // qk_n):
                                for kh2 in range(2):
                                    nc.tensor.matmul(
                                        st[64 * kh2 : 64 * kh2 + 64,
                                           j * qk_n : (j + 1) * qk_n],
                                        lhsT=kT_sb[
                                            :, c * P + 64 * kh2 : c * P + 64 * kh2 + 64
                                        ],
                                        rhs=qT_sb[:, q0 + j * qk_n : q0 + (j + 1) * qk_n],
                                        start=True,
                                        stop=True,
                                    )
                        pt = pt_pool.tile(
                            [P, span], DT16, tag="pt", name=f"pti{hp + i}_{qh}_{c}"
                        )
                        nc.scalar.activation(
                            pt[:], st[:], mybir.ActivationFunctionType.Exp
                        )
                        if not MASK_OFF:
                            nc.vector.tensor_mul(
                                pt[:], pt[:], maskT_sb[c][:, q0 : q0 + span]
                            )
                        pts.append(pt)
                    pend.append((c, pts))
                    if len(pend) > PV_DELAY:
                        cc, ppts = pend.pop(0)
                        emit_pv_i(0, cc, ppts[0])
                        emit_pv_i(1, cc, ppts[1])
                for cc, ppts in pend:
                    emit_pv_i(0, cc, ppts[0])
                    emit_pv_i(1, cc, ppts[1])
                for i in range(2):
                    if PV_OFF:
                        nc.gpsimd.memset(outs[i][:, q0 : q0 + span], 0.0)
                    else:
                        nc.vector.tensor_copy(outs[i][:, q0 : q0 + span], accs[i][:])
            for i in range(2):
                (nc.gpsimd if COPY_GPS else nc.sync).dma_start(o[hp + i], outs[i][:])

        def _heads(maskT_sb, ios):
          if IVL2:
              _heads_ivl2(maskT_sb, ios)
              return
          for h in range(HPC):
            if ios is not None and h < len(ios):
                qT_sb, kT_sb, vp_sb = ios[h]
            else:
                qT_sb, kT_sb, vp_sb = load_io(h)

            if QK_QUAD:
                _head_quad(h, qT_sb, kT_sb, vp_sb, maskT_sb)
                continue

            acc = None
            if not PV_OFF:
                acc = acc_psum.tile([D + 1, S], F32, tag="acc", name=f"acc{h}")

            def fill():
                # dummy weight loads: zero-latency in the cost model, but they
                # keep the PE pipeline occupied across short dependency stalls
                # so the p-state ramp (full speed only after ~3us of
                # uninterrupted execution) is not reset.
                for _ in range(LW_FILL):
                    nc.tensor.ldweights(qT_sb[0:64, 0:LW_W])

            def emit_pv(c, pts):
                if PV_OFF:
                    return
                for sp in range(spans):
                    fill()
                    for qs in range(span // 512):
                        q0 = sp * span + qs * 512
                        nc.tensor.matmul(
                            acc[:, q0 : q0 + 512],
                            lhsT=vp_sb[:, c, :],
                            rhs=pts[sp][:, qs * 512 : (qs + 1) * 512],
                            start=(c == 0),
                            stop=(c == NCHUNK - 1),
                        )

            def emit_pv_span(c, pt_sp, sp):
                if PV_OFF:
                    return
                fill()
                for qs in range(span // 512):
                    q0 = sp * span + qs * 512
                    nc.tensor.matmul(
                        acc[:, q0 : q0 + 512],
                        lhsT=vp_sb[:, c, :],
                        rhs=pt_sp[:, qs * 512 : (qs + 1) * 512],
                        start=(c == 0),
                        stop=(c == NCHUNK - 1),
                    )

            pending = []  # [(chunk, [pt tiles per span])] awaiting PV emission
            for c in range(NCHUNK):
                # with QK_PACK, chunk c runs on PE rows 0-63 (tile T0) and
                # chunk c^1 on rows 64-127 (tile T8), concurrently
                r0 = 64 * (c % 2) if QK_PACK else 0
                pts = []
                ptw = None
                if MASK_WIDE:
                    ptw = pt_pool.tile(
                        [P, S], DT16, tag="pt", name=f"ptw{h}_{c}", bufs=3
                    )
                for sp in range(spans):
                    st = qk_psum.tile([P, span], score_dt, tag="st", name=f"st{h}_{c}_{sp}")
                    fill()
                    if QK_DUP and not QK_OFF:
                        # two col-tile matmuls (out partitions 0-63 / 64-127)
                        # per 512-slice with K=128 duplicated rows.
                        # QK_KH_MAJOR pairs same-stationary matmuls
                        # back-to-back (possible weight-load elision);
                        # otherwise tile_position alternates (load ping-pong).
                        if QK_KH_MAJOR:
                            order = [(j, kh) for kh in range(2)
                                     for j in range(span // qk_n)]
                        else:
                            order = [(j, kh) for j in range(span // qk_n)
                                     for kh in range(2)]
                        for j, kh in order:
                            q0 = sp * span + j * qk_n
                            nc.tensor.matmul(
                                st[64 * kh : 64 * kh + 64,
                                   j * qk_n : (j + 1) * qk_n],
                                lhsT=kT_sb[:, c * P + 64 * kh : c * P + 64 * kh + 64],
                                rhs=qT_sb[:, q0 : q0 + qk_n],
                                start=True,
                                stop=True,
                            )
                    elif not QK_OFF:
                        for j in range(span // qk_n):
                            q0 = sp * span + j * qk_n
                            nc.tensor.matmul(
                                st[:, j * qk_n : (j + 1) * qk_n],
                                lhsT=kT_sb[r0 : r0 + 64, c * P : (c + 1) * P],
                                rhs=qT_sb[r0 : r0 + 64, q0 : q0 + qk_n],
                                start=True,
                                stop=True,
                            )
                    if MASK_WIDE:
                        pt = ptw[:, sp * span : (sp + 1) * span]
                    else:
                        pt = pt_pool.tile([P, span], DT16, tag="pt", name=f"pt{h}_{c}_{sp}")
                    nc.scalar.activation(pt[:], st[:], mybir.ActivationFunctionType.Exp)
                    if MASK_WIDE:
                        pts.append(pt)
                        if sp == spans - 1 and not MASK_OFF:
                            nc.vector.tensor_mul(ptw[:], ptw[:], maskT_sb[c][:])
                        continue
                    if not MASK_OFF:
                        if MASK_SEP:
                            ptm = pt_pool.tile(
                                [P, span], DT16, tag="ptm", name=f"ptm{h}_{c}_{sp}"
                            )
                            nc.vector.tensor_mul(
                                ptm[:], pt[:], maskT_sb[c][:, sp * span : (sp + 1) * span]
                            )
                            pt = ptm
                        else:
                            eng = nc.gpsimd if (MASK_GPS and sp % 2 == 1) else nc.vector
                            eng.tensor_mul(
                                pt[:], pt[:], maskT_sb[c][:, sp * span : (sp + 1) * span]
                            )
                    pts.append(pt)
                    if PV_ILV and pending:
                        emit_pv_span(pending[0][0], pending[0][1][sp], sp)
                pending.append((c, pts))
                if len(pending) > PV_DELAY:
                    done = pending.pop(0)
                    if not PV_ILV:
                        emit_pv(*done)
            for item in pending:
                if PV_ILV:
                    for sp in range(spans):
                        emit_pv_span(item[0], item[1][sp], sp)
                else:
                    emit_pv(*item)
            out_sb = out_pool.tile([D + 1, S], F32, tag="out", name=f"out_sb{h}")
            if PV_OFF:
                nc.gpsimd.memset(out_sb[:], 0.0)
            else:
                # GPSIMD cannot touch PSUM, so the acc drain stays on DVE;
                # only the HBM store's descriptor-gen moves to the idle Pool
                # queue (COPY_GPS) to keep SP free for mask/io dispatch.
                # Per-span copies so the next head's first PV matmuls (which
                # reuse this psum tile) only wait for the span they touch.
                for sp in range(spans):
                    nc.vector.tensor_copy(
                        out_sb[:, sp * span : (sp + 1) * span],
                        acc[:, sp * span : (sp + 1) * span],
                    )
            (nc.gpsimd if COPY_GPS else nc.sync).dma_start(o[h], out_sb[:])

        if loop_reps is None:
            body()
        else:
            if MASK_DMA_ONCE:
                hoisted[0] = load_mask()
            with tc.For_i(0, loop_reps, 1) as _i:
                body(_i)

    nc.compile()
    return nc


def _get_nc():
    if "nc" not in _CACHE:
        _CACHE["nc"] = _build_nc()
    return _CACHE["nc"]


def _prep_inputs(q, k, v, mask):
    """Host-side shard + layout prep. Returns one input map per core."""
    np16 = np.float16 if PREC == "fp16" else ml_dtypes.bfloat16
    qk_np_dt = np.float32 if QK_DT in ("f32", "f32r") else np16
    q = np.asarray(q, dtype=np.float32)
    k = np.asarray(k, dtype=np.float32)
    v = np.asarray(v, dtype=np.float32)
    mask = np.asarray(mask)

    # [B,H,S,D] -> [B*H, ...]
    qf = q.reshape(B * H, S, D)
    kf = k.reshape(B * H, S, D)
    vf = v.reshape(B * H, S, D)

    # transposed layouts; fold the 1/sqrt(D) scale into q before rounding
    qscale = np.float32(D) ** -0.5
    if QK_DUP or IVL2:
        # rows duplicated to contract over K=128: each product counted twice,
        # so halve the score scale.
        qscale = qscale / 2
    qTf = np.ascontiguousarray(np.transpose(qf * qscale, (0, 2, 1))).astype(qk_np_dt)  # [BH, 64, S]
    kTf = np.ascontiguousarray(np.transpose(kf, (0, 2, 1))).astype(qk_np_dt)
    if QK_PACK or QK_DUP or QK_QUAD or IVL2:
        # duplicate rows: QK_PACK -> chunk pairs on PE row-tiles T0/T8;
        # QK_DUP -> K=128 contraction for the col-tile form; QK_QUAD ->
        # valid operands at base partition 64 for row-tile quadrants
        # (each matmul still contracts K=64, so no extra scaling).
        qTf = np.concatenate([qTf, qTf], axis=1)  # [BH, 128, S]
        kTf = np.concatenate([kTf, kTf], axis=1)
    ones = np.ones((B * H, S, 1), np.float32)
    vpf = np.concatenate([vf, ones], axis=2).astype(np16)  # [BH, S, 65]
    maskT = np.ascontiguousarray(mask[0, 0].T).astype(np16)  # [S, S]

    in_maps = []
    for ci in range(N_CORES):
        sl = slice(ci * HPC, (ci + 1) * HPC)
        in_maps.append(
            {
                "qT": np.ascontiguousarray(qTf[sl]),
                "kT": np.ascontiguousarray(kTf[sl]),
                "vp": np.ascontiguousarray(vpf[sl]),
                "maskT": maskT,
            }
        )
    return in_maps


def kernel(q, k, v, mask):
    global LAST_RESULT
    from concourse import bass_utils

    nc = _get_nc()
    in_maps = _prep_inputs(q, k, v, mask)
    res = bass_utils.run_bass_kernel_spmd(
        nc, in_maps, core_ids=list(range(N_CORES))
    )
    LAST_RESULT = res

    out = np.empty((B * H, S, D), np.float32)
    for ci in range(N_CORES):
        oc = res.results[ci]["o"]  # [HPC, 65, S] f32
        num = oc[:, :D, :]  # (P@V)^T
        den = oc[:, D : D + 1, :]  # l
        out[ci * HPC : (ci + 1) * HPC] = np.transpose(num / den, (0, 2, 1))
    return out.reshape(B, H, S, D)

